# revision 20
# baseline (speedup 1.0000x reference)
"""Trainium2 Bass kernel for nn_CDDLinear (class-discrepancy distances).

Strategy (8 NeuronCores, SPMD):
  - The heavy tensor is queue (500, 256, 400) f32; only the first
    queue_size[c] of the 400 columns of each class contribute.  Classes are
    sorted by queue_size (desc) and dealt round-robin onto (slot, core):
    slot j on core i holds the class with rank j*8+i.  Every core then has
    63 class slots whose compile-time column count Lmax[j] = max queue_size
    within the rank group -- identical across cores (perfect load balance)
    and only ~1-2% above the true valid-column count.  The host zero-fills
    each class's columns in [queue_size[c], Lmax[j]) so the device can do
    plain prefix-sum reductions with no masking pass.
  - Queue slots are further packed into a handful of *group* DMAs (DP chooses
    the grouping: each DMA costs ~1.5us fixed on the serial HWDGE queue, so
    few big transfers win; padding each group to its max column count costs
    bytes).  One DVE tensor_reduce per group does the masked sums.
  - x/y batch tensors load via the Activation HWDGE queue so the two DMA
    queues overlap; batch matmuls y_shard.T @ [x | 1] run on the PE (the
    ones column folds sum(y) into the same matmul).
  - Means are assembled with a PE transpose; one 8-core AllGather ships
    [mean_src^T | n2(+pad bias)]; a gram matmul with two extra contraction
    rows produces n2[c] + n2[c'] - 2*gram directly in PSUM; clamp+sqrt+
    row-reduce gives per-class intra distances and inter row-sums.
  - Host gathers per-(core,slot) values, drops padding slots, divides by
    500 / 500^2.
"""

import os
import types
import sys

import numpy as np

C, F, K, N = 500, 256, 400, 4096
NCORES = 8
CL = 63          # class slots per core (63 * 8 = 504 >= 500)
CPAD = CL * NCORES
EPS = 1e-8
NEG_BIG = -1.0e30  # n2 bias for padding slots -> distance collapses to 1e-6

# DMA cost model for the group-size DP: ~1.5us fixed per DMA on the serial
# HW queue; HBM 358 GB/s = 358 B/ns -> ns per queue column (128*2 rows * 4B)
DMA_FIXED_NS = 1500.0
NS_PER_COL = (128 * 2 * 4) / 358.0
MAX_GROUP_ELEMS = 6400  # 2*jg*Lg f32 per partition (25.6KB) SBUF cap per tile

LAST_EXEC_NS = None  # set when CDD_KERNEL_TRACE=1


def _install_ntff_shim():
    """antenv.axon_hooks is missing in this image; inject it so
    run_bass_kernel_spmd(trace=True) can reach the NTFF profiler."""
    try:
        import antenv.axon_hooks  # noqa: F401
        return
    except ImportError:
        pass
    try:
        from trn_agent_boot.trn_boot import _ntff_profile_via_ctypes
        hook = _ntff_profile_via_ctypes("/opt/axon/libaxon_pjrt.so")
    except Exception:
        hook = None
    mod = types.ModuleType("antenv.axon_hooks")
    mod._hook = hook
    mod.get_axon_ntff_profile_hook = lambda: mod._hook
    mod.set_axon_ntff_profile_hook = lambda h: setattr(mod, "_hook", h)
    sys.modules["antenv.axon_hooks"] = mod
    import antenv
    antenv.axon_hooks = mod


def _plan(queue_size):
    """Class assignment, per-slot lengths, and DMA grouping."""
    qs = np.asarray(queue_size).astype(np.int64)
    order = np.argsort(-qs, kind="stable")
    assign = np.full((CL, NCORES), -1, dtype=np.int64)
    for r in range(C):
        j, i = divmod(r, NCORES)
        assign[j, i] = order[r]
    lmax = np.zeros(CL, dtype=np.int64)
    for j in range(CL):
        sizes = [qs[c] for c in assign[j] if c >= 0]
        lmax[j] = max(sizes) if sizes else 0

    # DP: partition slots [0..CL) (lmax non-increasing) into contiguous groups
    # minimizing sum of (fixed + bytes(group)), bytes = 2*jg*Lg cols padded to
    # the group's max (= first slot's lmax).
    INF = float("inf")
    best = [INF] * (CL + 1)
    prev = [0] * (CL + 1)
    best[0] = 0.0
    for e in range(1, CL + 1):
        for s in range(e):
            jg = e - s
            Lg = int(lmax[s])
            if Lg == 0:
                cost = 0.0 if jg == CL - s else INF  # tail of zeros: free, one group
            else:
                if 2 * jg * Lg > MAX_GROUP_ELEMS:
                    continue
                cost = DMA_FIXED_NS + jg * Lg * NS_PER_COL
            if best[s] + cost < best[e]:
                best[e] = best[s] + cost
                prev[e] = s
    groups = []
    e = CL
    while e > 0:
        s = prev[e]
        groups.append((s, e, int(lmax[s])))
        e = s
    groups.reverse()
    return assign, lmax, groups


def _build_program(lmax, groups):
    import concourse.bacc as bacc
    import concourse.tile as tile
    from concourse import mybir
    from concourse.masks import make_identity

    skip_ag = os.environ.get("CDD_SKIP_AG", "0") == "1"

    f32 = mybir.dt.float32
    nc = bacc.Bacc(None, target_bir_lowering=False, num_devices=NCORES)

    qshard = nc.dram_tensor("qshard", [CL, F, K], f32, kind="ExternalInput")
    ysrc = nc.dram_tensor("ysrc", [N, CL], f32, kind="ExternalInput")
    ytgt = nc.dram_tensor("ytgt", [N, CL], f32, kind="ExternalInput")
    xsrc = nc.dram_tensor("xsrc", [N, F], f32, kind="ExternalInput")
    xtgt = nc.dram_tensor("xtgt", [N, F], f32, kind="ExternalInput")
    numcol = nc.dram_tensor("numcol", [CL, 1], f32, kind="ExternalInput")
    invbias = nc.dram_tensor("invbias", [CL, 1], f32, kind="ExternalInput")
    out_vals = nc.dram_tensor("out_vals", [CL, 2], f32, kind="ExternalOutput")

    NT = N // 128  # 32 batch k-tiles
    F1 = F + 1     # x augmented with a ones column
    CL2 = 2 * CL

    with tile.TileContext(nc) as tc:
        with (
            tc.tile_pool(name="sing", bufs=1) as sing,
            tc.tile_pool(name="qpool", bufs=2) as qpool,
            tc.tile_pool(name="psum", bufs=1, space="PSUM") as psum,
            tc.tile_pool(name="dram", bufs=1, space="DRAM") as dram,
        ):
            # ---- persistent SBUF tiles ----
            xs = sing.tile([128, NT, F1], f32, tag="xs")
            xt = sing.tile([128, NT, F1], f32, tag="xt")
            ys = sing.tile([128, NT, CL], f32, tag="ys")
            yt = sing.tile([128, NT, CL], f32, tag="yt")
            # qsum columns ordered (f-half h, slot j): col h*CL+j
            qsum = sing.tile([128, CL2], f32, tag="qsum")
            numc = sing.tile([CL, 1], f32, tag="numc")
            invb = sing.tile([CL, 1], f32, tag="invb")
            ident = sing.tile([128, 128], f32, tag="ident")
            epsT = sing.tile([CL, 1], f32, tag="epsT")
            eps12 = sing.tile([CL, 1], f32, tag="eps12")
            m2 = sing.tile([CL, 1], f32, tag="m2")
            ov = sing.tile([CL, 2], f32, tag="ov")

            make_identity(nc, ident[:, :])
            nc.vector.memset(epsT, EPS)
            nc.vector.memset(eps12, 1e-12)
            nc.vector.memset(m2, -2.0)

            # ---- queue group DMAs (SP HWDGE queue) + DVE reduces ----
            # qshard[j] is a contiguous [256, 400] block = 2 halves of
            # 128*400; merged (j,h) dim has uniform stride 128*400.
            qflat = qshard[:, :, :].rearrange("j (h p) k -> p (j h) k", h=2)
            # strided view of qsum as [p, slot, half] (half stride = CL)
            qsum_jh = qsum.rearrange("p (h c) -> p c h", h=2)
            for (s, e, Lg) in groups:
                jg = e - s
                if Lg == 0:
                    nc.vector.memset(qsum_jh[:, s:e, :], 0.0)
                    continue
                qt_t = qpool.tile([128, 2 * jg, Lg], f32, tag="qt")
                nc.sync.dma_start(
                    out=qt_t, in_=qflat[:, 2 * s : 2 * e, 0:Lg]
                )
                nc.vector.tensor_reduce(
                    out=qsum_jh[:, s:e, :], in_=qt_t,
                    axis=mybir.AxisListType.X, op=mybir.AluOpType.add,
                )

            # ---- x/y loads (Activation HWDGE queue, overlaps SP queue) ----
            nc.scalar.dma_start(
                out=ys, in_=ysrc[:, :].rearrange("(t p) c -> p t c", p=128)
            )
            nc.scalar.dma_start(
                out=xs[:, :, 0:F], in_=xsrc[:, :].rearrange("(t p) f -> p t f", p=128)
            )
            nc.scalar.dma_start(
                out=yt, in_=ytgt[:, :].rearrange("(t p) c -> p t c", p=128)
            )
            nc.scalar.dma_start(
                out=xt[:, :, 0:F], in_=xtgt[:, :].rearrange("(t p) f -> p t f", p=128)
            )
            nc.vector.memset(xs[:, :, F:F1], 1.0)
            nc.vector.memset(xt[:, :, F:F1], 1.0)
            nc.sync.dma_start(out=numc, in_=numcol[:, :])
            nc.sync.dma_start(out=invb, in_=invbias[:, :])

            # ---- batch matmuls: ps_s[c, f'] = sum_n y[n, c] * [x | 1][n, f'] ----
            ps_s = psum.tile([CL, F1], f32, tag="ps_s")
            ps_t = psum.tile([CL, F1], f32, tag="ps_t")
            for t in range(NT):
                nc.tensor.matmul(
                    ps_s[:, :], lhsT=ys[:, t, :], rhs=xs[:, t, :],
                    start=(t == 0), stop=(t == NT - 1),
                )
            for t in range(NT):
                nc.tensor.matmul(
                    ps_t[:, :], lhsT=yt[:, t, :], rhs=xt[:, t, :],
                    start=(t == 0), stop=(t == NT - 1),
                )

            # ---- combine into means ----
            total_s = sing.tile([CL, 1], f32, tag="total_s")
            recip_s = sing.tile([CL, 1], f32, tag="recip_s")
            total_t = sing.tile([CL, 1], f32, tag="total_t")
            recip_t = sing.tile([CL, 1], f32, tag="recip_t")
            nc.vector.tensor_scalar(
                out=total_s, in0=ps_s[:, F:F1], scalar1=numc, scalar2=epsT,
                op0=mybir.AluOpType.add, op1=mybir.AluOpType.add,
            )
            nc.vector.reciprocal(out=recip_s, in_=total_s)
            nc.vector.tensor_scalar(
                out=total_t, in0=ps_t[:, F:F1], scalar1=epsT, scalar2=None,
                op0=mybir.AluOpType.add,
            )
            nc.vector.reciprocal(out=recip_t, in_=total_t)

            # qsum [128f, (h c)] -> qsum_cf [63c, 256f] via two PE transposes
            ps_q = psum.tile([CL, 2, 128], f32, tag="ps_q")
            for h in range(2):
                nc.tensor.transpose(
                    out=ps_q[:, h, :], in_=qsum[:, h * CL : (h + 1) * CL],
                    identity=ident[:, :],
                )
            qsum_cf = sing.tile([CL, F], f32, tag="qsum_cf")
            nc.scalar.copy(out=qsum_cf, in_=ps_q.rearrange("c h p -> c (h p)"))

            ms = sing.tile([CL, F], f32, tag="ms")
            mt = sing.tile([CL, F], f32, tag="mt")
            nc.vector.tensor_tensor(
                out=ms, in0=qsum_cf, in1=ps_s[:, 0:F], op=mybir.AluOpType.add
            )
            nc.vector.tensor_scalar_mul(out=ms, in0=ms, scalar1=recip_s)
            nc.vector.tensor_scalar_mul(out=mt, in0=ps_t[:, 0:F], scalar1=recip_t)

            # ---- per-class n2 / dot / intra ----
            # (tensor_tensor_reduce crashes the worker on this toolchain;
            #  use plain mult + reduce)
            sq = sing.tile([CL, F], f32, tag="sq")
            n2s = sing.tile([CL, 1], f32, tag="n2s")
            n2t = sing.tile([CL, 1], f32, tag="n2t")
            dot = sing.tile([CL, 1], f32, tag="dot")
            for pair_in0, pair_in1, pair_acc in (
                (ms, ms, n2s), (mt, mt, n2t), (ms, mt, dot)
            ):
                nc.vector.tensor_tensor(
                    out=sq, in0=pair_in0, in1=pair_in1, op=mybir.AluOpType.mult
                )
                nc.vector.tensor_reduce(
                    out=pair_acc, in_=sq,
                    axis=mybir.AxisListType.X, op=mybir.AluOpType.add,
                )
            ss = sing.tile([CL, 1], f32, tag="ss")
            intra_t = sing.tile([CL, 1], f32, tag="intra_t")
            nc.vector.tensor_tensor(out=ss, in0=n2s, in1=n2t, op=mybir.AluOpType.add)
            nc.vector.tensor_scalar(
                out=ss, in0=dot, scalar1=m2, scalar2=ss,
                op0=mybir.AluOpType.mult, op1=mybir.AluOpType.add,
            )
            nc.vector.tensor_scalar(
                out=ss, in0=ss, scalar1=eps12, scalar2=None, op0=mybir.AluOpType.max
            )
            nc.scalar.activation(
                out=intra_t, in_=ss, func=mybir.ActivationFunctionType.Sqrt
            )

            # n2 shipped to other cores carries the padding-slot bias
            n2ag = sing.tile([CL, 1], f32, tag="n2ag")
            nc.vector.tensor_tensor(out=n2ag, in0=n2s, in1=invb, op=mybir.AluOpType.add)

            # ---- mean_src^T for the gram matmul ----
            ps_m = psum.tile([128, 2, CL], f32, tag="ps_m")
            for h in range(2):
                nc.tensor.transpose(
                    out=ps_m[:, h, :], in_=ms[:, h * 128 : (h + 1) * 128],
                    identity=ident[0:CL, 0:CL],
                )
            lhsT = sing.tile([128, 2, CL], f32, tag="lhsT")
            msT = sing.tile([128, 2, CL], f32, tag="msT")
            for h in range(2):
                nc.scalar.mul(out=lhsT[:, h, :], in_=ps_m[:, h, :], mul=-2.0)
                nc.scalar.copy(out=msT[:, h, :], in_=ps_m[:, h, :])

            inter_rows = sing.tile([CL, 1], f32, tag="inter_rows")
            if skip_ag:
                nc.vector.memset(inter_rows, 0.0)
            else:
                ag_in = dram.tile([2 * 128 + 1, CL], f32)
                ag_out = dram.tile([NCORES, 2 * 128 + 1, CL], f32, addr_space="Shared")
                nc.sync.dma_start(
                    out=ag_in[0:256, :].rearrange("(h p) c -> p h c", h=2), in_=msT
                )
                nc.sync.dma_start(out=ag_in[256:257, :], in_=n2ag)
                nc.gpsimd.collective_compute(
                    "AllGather",
                    mybir.AluOpType.bypass,
                    replica_groups=[list(range(NCORES))],
                    ins=[ag_in.opt()],
                    outs=[ag_out.opt()],
                )

                rhs = sing.tile([128, 2, NCORES, CL], f32, tag="rhs")
                for h in range(2):
                    nc.sync.dma_start(
                        out=rhs[:, h, :, :],
                        in_=ag_out[:, h * 128 : (h + 1) * 128, :].rearrange(
                            "r p c -> p r c"
                        ),
                    )
                n2r = sing.tile([2, NCORES, CL], f32, tag="n2r")
                nc.vector.memset(n2r[:, :, :], 1.0)
                nc.sync.dma_start(out=n2r[1:2, :, :], in_=ag_out[:, 256, :])
                e2 = sing.tile([2, CL], f32, tag="e2")
                nc.vector.memset(e2[:, :], 1.0)
                nc.sync.dma_start(out=e2[0:1, :], in_=ag_in[256:257, :])

                # ---- gram matmul producing n2[c] + n2[c'] - 2*gram ----
                CA = NCORES * CL
                ps_g = psum.tile([CL, CA], f32, tag="ps_g")
                nc.tensor.matmul(
                    ps_g[:, :], lhsT=lhsT[:, 0, :],
                    rhs=rhs[:, 0, :, :].rearrange("p r c -> p (r c)"),
                    start=True, stop=False,
                )
                nc.tensor.matmul(
                    ps_g[:, :], lhsT=lhsT[:, 1, :],
                    rhs=rhs[:, 1, :, :].rearrange("p r c -> p (r c)"),
                    start=False, stop=False,
                )
                nc.tensor.matmul(
                    ps_g[:, :], lhsT=e2[:, :],
                    rhs=n2r.rearrange("two r c -> two (r c)"),
                    start=False, stop=True,
                )

                dist = sing.tile([CL, CA], f32, tag="dist")
                nc.vector.tensor_scalar(
                    out=dist, in0=ps_g[:, :], scalar1=eps12, scalar2=None,
                    op0=mybir.AluOpType.max,
                )
                nc.scalar.activation(
                    out=dist, in_=dist, func=mybir.ActivationFunctionType.Sqrt
                )
                nc.vector.tensor_reduce(
                    out=inter_rows, in_=dist,
                    axis=mybir.AxisListType.X, op=mybir.AluOpType.add,
                )

            # ---- output ----
            nc.vector.tensor_copy(out=ov[:, 0:1], in_=intra_t)
            nc.vector.tensor_copy(out=ov[:, 1:2], in_=inter_rows)
            nc.sync.dma_start(out=out_vals[:, :], in_=ov)

    nc.finalize()
    return nc


def kernel(src_x, tgt_x, src_y, tgt_y, queue, queue_size):
    global LAST_EXEC_NS
    _install_ntff_shim()
    from concourse.bass_utils import run_bass_kernel_spmd

    src_x = np.ascontiguousarray(np.asarray(src_x, dtype=np.float32))
    tgt_x = np.ascontiguousarray(np.asarray(tgt_x, dtype=np.float32))
    src_y = np.asarray(src_y, dtype=np.float32)
    tgt_y = np.asarray(tgt_y, dtype=np.float32)
    queue = np.asarray(queue, dtype=np.float32)
    qs = np.asarray(queue_size).astype(np.int64)

    assign, lmax, groups = _plan(qs)
    nc = _build_program(lmax, groups)

    in_maps = []
    for i in range(NCORES):
        cols = assign[:, i]  # global class per slot, -1 for padding
        qshard = np.zeros((CL, F, K), dtype=np.float32)
        ysh = np.zeros((N, CL), dtype=np.float32)
        tsh = np.zeros((N, CL), dtype=np.float32)
        numv = np.zeros((CL, 1), dtype=np.float32)
        ivb = np.zeros((CL, 1), dtype=np.float32)
        for j, c in enumerate(cols):
            if c < 0:
                ivb[j, 0] = NEG_BIG
                continue
            L = int(qs[c])
            qshard[j, :, 0:L] = queue[c, :, 0:L]
            ysh[:, j] = src_y[:, c]
            tsh[:, j] = tgt_y[:, c]
            numv[j, 0] = float(L)
        in_maps.append(
            {
                "qshard": qshard,
                "ysrc": np.ascontiguousarray(ysh),
                "ytgt": np.ascontiguousarray(tsh),
                "xsrc": src_x,
                "xtgt": tgt_x,
                "numcol": numv,
                "invbias": ivb,
            }
        )

    trace = os.environ.get("CDD_KERNEL_TRACE", "0") == "1"
    kwargs = {}
    if trace:
        kwargs["trace"] = True
        tdir = os.environ.get("CDD_KERNEL_TRACE_DIR")
        if tdir:
            os.makedirs(tdir, exist_ok=True)
            kwargs["tmpdir"] = tdir
    res = run_bass_kernel_spmd(
        nc, in_maps, core_ids=list(range(NCORES)), **kwargs
    )
    LAST_EXEC_NS = res.exec_time_ns

    intra_sum = 0.0
    inter_sum = 0.0
    for i in range(NCORES):
        ov = res.results[i]["out_vals"]  # [CL, 2]
        for j, c in enumerate(assign[:, i]):
            if c >= 0:
                intra_sum += float(ov[j, 0])
                inter_sum += float(ov[j, 1])
    intra = np.float32(intra_sum / C)
    inter = np.float32(inter_sum / (C * C))
    return intra, inter


# revision 21
# speedup vs baseline: 1.2591x; 1.2591x over previous
"""Trainium2 Bass kernel for nn_CDDLinear (class-discrepancy distances).

Strategy (8 NeuronCores, SPMD):
  - The heavy tensor is queue (500, 256, 400) f32; only the first
    queue_size[c] of the 400 columns of each class contribute.  Classes are
    sorted by queue_size (desc) and dealt round-robin onto (slot, core):
    slot j on core i holds the class with rank j*8+i, so every core carries
    the same per-slot column count L[j] (perfect load balance).  The host
    packs each core's valid queue columns contiguously per feature row
    (qpack[f] = concat_j queue[c_j][f, 0:L_j]) so the device streams only
    valid bytes with ~48KB contiguous runs per partition -- full HBM rate --
    in a handful of chunk DMAs on the SP HWDGE queue.  One DVE tensor_reduce
    per (class-slot) computes the masked sums.
  - x/y batch tensors are host-pre-tiled into the exact SBUF layout
    [128, 32, w] (contiguous per partition) and loaded via the Activation
    HWDGE queue so the two DMA queues overlap.  Batch matmuls
    y_shard.T @ [x | 1] run on the PE (the ones column folds sum(y) into the
    same matmul); inputs are optionally bf16 (fp32 PSUM accumulation).
  - Means are assembled with PE transposes; one 8-core AllGather ships
    [mean_src^T | n2(+pad bias)]; a gram matmul with two extra contraction
    rows produces n2[c] + n2[c'] - 2*gram directly in PSUM; clamp+sqrt+
    row-reduce gives per-class intra distances and inter row-sums.
  - Host gathers per-(core,slot) values, drops padding slots, divides by
    500 / 500^2.
"""

import os
import types
import sys

import numpy as np

C, F, K, N = 500, 256, 400, 4096
NCORES = 8
CL = 63          # class slots per core (63 * 8 = 504 >= 500)
EPS = 1e-8
NEG_BIG = -1.0e30  # n2 bias for padding slots -> distance collapses to 1e-6
CHUNK_COLS = 3072  # target packed columns per queue DMA (24KB/partition f32)

LAST_EXEC_NS = None  # set when CDD_KERNEL_TRACE=1


def _install_ntff_shim():
    """antenv.axon_hooks is missing in this image; inject it so
    run_bass_kernel_spmd(trace=True) can reach the NTFF profiler."""
    try:
        import antenv.axon_hooks  # noqa: F401
        return
    except ImportError:
        pass
    try:
        from trn_agent_boot.trn_boot import _ntff_profile_via_ctypes
        hook = _ntff_profile_via_ctypes("/opt/axon/libaxon_pjrt.so")
    except Exception:
        hook = None
    mod = types.ModuleType("antenv.axon_hooks")
    mod._hook = hook
    mod.get_axon_ntff_profile_hook = lambda: mod._hook
    mod.set_axon_ntff_profile_hook = lambda h: setattr(mod, "_hook", h)
    sys.modules["antenv.axon_hooks"] = mod
    import antenv
    antenv.axon_hooks = mod


def _mm_dtype_np():
    import ml_dtypes
    return (
        np.float32
        if os.environ.get("CDD_MM_DTYPE", "bf16") == "f32"
        else ml_dtypes.bfloat16
    )


def _plan(queue_size):
    """Class assignment, per-slot lengths, offsets, and chunking."""
    qs = np.asarray(queue_size).astype(np.int64)
    order = np.argsort(-qs, kind="stable")
    assign = np.full((CL, NCORES), -1, dtype=np.int64)
    for r in range(C):
        j, i = divmod(r, NCORES)
        assign[j, i] = order[r]
    lmax = np.zeros(CL, dtype=np.int64)
    for j in range(CL):
        sizes = [int(qs[c]) for c in assign[j] if c >= 0]
        lmax[j] = max(sizes) if sizes else 0
    offs = np.concatenate([[0], np.cumsum(lmax)])  # slot j occupies [offs[j], offs[j+1])
    tot = int(offs[-1])
    # contiguous chunks of slots, each <= CHUNK_COLS packed columns
    chunks = []  # (slot_start, slot_end, col_start, col_end)
    s = 0
    while s < CL and lmax[s] > 0:
        e = s
        while e < CL and lmax[e] > 0 and (offs[e + 1] - offs[s]) <= CHUNK_COLS:
            e += 1
        if e == s:
            e = s + 1  # single oversized slot (can't happen: L<=400<CHUNK)
        chunks.append((s, e, int(offs[s]), int(offs[e])))
        s = e
    zero_start = s  # slots [zero_start, CL) have L == 0
    return assign, lmax, offs, tot, chunks, zero_start


def _build_program(lmax, offs, tot, chunks, zero_start):
    import concourse.bacc as bacc
    import concourse.tile as tile
    from concourse import mybir
    from concourse.masks import make_identity

    skip_ag = os.environ.get("CDD_SKIP_AG", "0") == "1"
    mm_f32 = os.environ.get("CDD_MM_DTYPE", "bf16") == "f32"

    f32 = mybir.dt.float32
    mmdt = f32 if mm_f32 else mybir.dt.bfloat16
    nc = bacc.Bacc(None, target_bir_lowering=False, num_devices=NCORES)

    NT = N // 128  # 32 batch k-tiles
    F1 = F + 1     # x augmented with a ones column

    qpack = nc.dram_tensor("qpack", [F, tot], f32, kind="ExternalInput")
    ysrc = nc.dram_tensor("ysrc", [128, NT, CL], mmdt, kind="ExternalInput")
    ytgt = nc.dram_tensor("ytgt", [128, NT, CL], mmdt, kind="ExternalInput")
    xsrc = nc.dram_tensor("xsrc", [128, NT, F], mmdt, kind="ExternalInput")
    xtgt = nc.dram_tensor("xtgt", [128, NT, F], mmdt, kind="ExternalInput")
    numcol = nc.dram_tensor("numcol", [CL, 1], f32, kind="ExternalInput")
    invbias = nc.dram_tensor("invbias", [CL, 1], f32, kind="ExternalInput")
    out_vals = nc.dram_tensor("out_vals", [CL, 2], f32, kind="ExternalOutput")

    CL2 = 2 * CL

    with tile.TileContext(nc) as tc:
        with (
            tc.tile_pool(name="sing", bufs=1) as sing,
            tc.tile_pool(name="qpool", bufs=2) as qpool,
            tc.tile_pool(name="psum", bufs=1, space="PSUM") as psum,
            tc.tile_pool(name="dram", bufs=1, space="DRAM") as dram,
        ):
            # ---- persistent SBUF tiles ----
            xs = sing.tile([128, NT, F1], mmdt, tag="xs")
            xt = sing.tile([128, NT, F1], mmdt, tag="xt")
            ys = sing.tile([128, NT, CL], mmdt, tag="ys")
            yt = sing.tile([128, NT, CL], mmdt, tag="yt")
            # qsum columns ordered (f-half h, slot j): col h*CL+j
            qsum = sing.tile([128, CL2], f32, tag="qsum")
            numc = sing.tile([CL, 1], f32, tag="numc")
            invb = sing.tile([CL, 1], f32, tag="invb")
            ident = sing.tile([128, 128], f32, tag="ident")
            epsT = sing.tile([CL, 1], f32, tag="epsT")
            eps12 = sing.tile([CL, 1], f32, tag="eps12")
            m2 = sing.tile([CL, 1], f32, tag="m2")
            ov = sing.tile([CL, 2], f32, tag="ov")

            make_identity(nc, ident[:, :])
            nc.vector.memset(epsT, EPS)
            nc.vector.memset(eps12, 1e-12)
            nc.vector.memset(m2, -2.0)

            # ---- x/y loads (Activation HWDGE queue; host pre-tiled) ----
            nc.scalar.dma_start(out=ys, in_=ysrc[:, :, :])
            nc.scalar.dma_start(out=xs[:, :, 0:F], in_=xsrc[:, :, :])
            nc.scalar.dma_start(out=yt, in_=ytgt[:, :, :])
            nc.scalar.dma_start(out=xt[:, :, 0:F], in_=xtgt[:, :, :])
            nc.vector.memset(xs[:, :, F:F1], 1.0)
            nc.vector.memset(xt[:, :, F:F1], 1.0)
            nc.sync.dma_start(out=numc, in_=numcol[:, :])
            nc.sync.dma_start(out=invb, in_=invbias[:, :])

            # ---- packed queue chunk DMAs (SP HWDGE queue) + DVE reduces ----
            # qpack rows: f = h*128 + p; strided qsum view [p, slot, half]
            qsum_jh = qsum.rearrange("p (h c) -> p c h", h=2)
            if zero_start < CL:
                nc.vector.memset(qsum_jh[:, zero_start:CL, :], 0.0)
            qrows = qpack[:, :].rearrange("(h p) k -> p h k", h=2)
            for (s, e, c0, c1) in chunks:
                qt_t = qpool.tile([128, 2, c1 - c0], f32, tag="qt")
                nc.sync.dma_start(out=qt_t, in_=qrows[:, :, c0:c1])
                for j in range(s, e):
                    a = int(offs[j]) - c0
                    L = int(lmax[j])
                    nc.vector.tensor_reduce(
                        out=qsum_jh[:, j, :], in_=qt_t[:, :, a : a + L],
                        axis=mybir.AxisListType.X, op=mybir.AluOpType.add,
                    )

            # ---- batch matmuls: ps_s[c, f'] = sum_n y[n, c] * [x | 1][n, f'] ----
            ps_s = psum.tile([CL, F1], f32, tag="ps_s")
            ps_t = psum.tile([CL, F1], f32, tag="ps_t")
            for t in range(NT):
                nc.tensor.matmul(
                    ps_s[:, :], lhsT=ys[:, t, :], rhs=xs[:, t, :],
                    start=(t == 0), stop=(t == NT - 1),
                )
            for t in range(NT):
                nc.tensor.matmul(
                    ps_t[:, :], lhsT=yt[:, t, :], rhs=xt[:, t, :],
                    start=(t == 0), stop=(t == NT - 1),
                )

            # ---- combine into means ----
            total_s = sing.tile([CL, 1], f32, tag="total_s")
            recip_s = sing.tile([CL, 1], f32, tag="recip_s")
            total_t = sing.tile([CL, 1], f32, tag="total_t")
            recip_t = sing.tile([CL, 1], f32, tag="recip_t")
            nc.vector.tensor_scalar(
                out=total_s, in0=ps_s[:, F:F1], scalar1=numc, scalar2=epsT,
                op0=mybir.AluOpType.add, op1=mybir.AluOpType.add,
            )
            nc.vector.reciprocal(out=recip_s, in_=total_s)
            nc.vector.tensor_scalar(
                out=total_t, in0=ps_t[:, F:F1], scalar1=epsT, scalar2=None,
                op0=mybir.AluOpType.add,
            )
            nc.vector.reciprocal(out=recip_t, in_=total_t)

            # qsum [128f, (h c)] -> qsum_cf [63c, 256f] via two PE transposes
            ps_q = psum.tile([CL, 2, 128], f32, tag="ps_q")
            for h in range(2):
                nc.tensor.transpose(
                    out=ps_q[:, h, :], in_=qsum[:, h * CL : (h + 1) * CL],
                    identity=ident[:, :],
                )
            qsum_cf = sing.tile([CL, F], f32, tag="qsum_cf")
            nc.scalar.copy(out=qsum_cf, in_=ps_q.rearrange("c h p -> c (h p)"))

            ms = sing.tile([CL, F], f32, tag="ms")
            mt = sing.tile([CL, F], f32, tag="mt")
            nc.vector.tensor_tensor(
                out=ms, in0=qsum_cf, in1=ps_s[:, 0:F], op=mybir.AluOpType.add
            )
            nc.vector.tensor_scalar_mul(out=ms, in0=ms, scalar1=recip_s)
            nc.vector.tensor_scalar_mul(out=mt, in0=ps_t[:, 0:F], scalar1=recip_t)

            # ---- per-class n2 / dot / intra ----
            # (tensor_tensor_reduce crashes the worker on this toolchain;
            #  use plain mult + reduce)
            sq = sing.tile([CL, F], f32, tag="sq")
            n2s = sing.tile([CL, 1], f32, tag="n2s")
            n2t = sing.tile([CL, 1], f32, tag="n2t")
            dot = sing.tile([CL, 1], f32, tag="dot")
            for pair_in0, pair_in1, pair_acc in (
                (ms, ms, n2s), (mt, mt, n2t), (ms, mt, dot)
            ):
                nc.vector.tensor_tensor(
                    out=sq, in0=pair_in0, in1=pair_in1, op=mybir.AluOpType.mult
                )
                nc.vector.tensor_reduce(
                    out=pair_acc, in_=sq,
                    axis=mybir.AxisListType.X, op=mybir.AluOpType.add,
                )
            ss = sing.tile([CL, 1], f32, tag="ss")
            intra_t = sing.tile([CL, 1], f32, tag="intra_t")
            nc.vector.tensor_tensor(out=ss, in0=n2s, in1=n2t, op=mybir.AluOpType.add)
            nc.vector.tensor_scalar(
                out=ss, in0=dot, scalar1=m2, scalar2=ss,
                op0=mybir.AluOpType.mult, op1=mybir.AluOpType.add,
            )
            nc.vector.tensor_scalar(
                out=ss, in0=ss, scalar1=eps12, scalar2=None, op0=mybir.AluOpType.max
            )
            nc.scalar.activation(
                out=intra_t, in_=ss, func=mybir.ActivationFunctionType.Sqrt
            )

            # n2 shipped to other cores carries the padding-slot bias
            n2ag = sing.tile([CL, 1], f32, tag="n2ag")
            nc.vector.tensor_tensor(out=n2ag, in0=n2s, in1=invb, op=mybir.AluOpType.add)

            # ---- mean_src^T for the gram matmul ----
            ps_m = psum.tile([128, 2, CL], f32, tag="ps_m")
            for h in range(2):
                nc.tensor.transpose(
                    out=ps_m[:, h, :], in_=ms[:, h * 128 : (h + 1) * 128],
                    identity=ident[0:CL, 0:CL],
                )
            lhsT = sing.tile([128, 2, CL], f32, tag="lhsT")
            msT = sing.tile([128, 2, CL], f32, tag="msT")
            for h in range(2):
                nc.scalar.mul(out=lhsT[:, h, :], in_=ps_m[:, h, :], mul=-2.0)
                nc.scalar.copy(out=msT[:, h, :], in_=ps_m[:, h, :])

            inter_rows = sing.tile([CL, 1], f32, tag="inter_rows")
            if skip_ag:
                nc.vector.memset(inter_rows, 0.0)
            else:
                ag_in = dram.tile([2 * 128 + 1, CL], f32)
                ag_out = dram.tile([NCORES, 2 * 128 + 1, CL], f32, addr_space="Shared")
                nc.sync.dma_start(
                    out=ag_in[0:256, :].rearrange("(h p) c -> p h c", h=2), in_=msT
                )
                nc.sync.dma_start(out=ag_in[256:257, :], in_=n2ag)
                nc.gpsimd.collective_compute(
                    "AllGather",
                    mybir.AluOpType.bypass,
                    replica_groups=[list(range(NCORES))],
                    ins=[ag_in.opt()],
                    outs=[ag_out.opt()],
                )

                rhs = sing.tile([128, 2, NCORES, CL], f32, tag="rhs")
                for h in range(2):
                    nc.sync.dma_start(
                        out=rhs[:, h, :, :],
                        in_=ag_out[:, h * 128 : (h + 1) * 128, :].rearrange(
                            "r p c -> p r c"
                        ),
                    )
                n2r = sing.tile([2, NCORES, CL], f32, tag="n2r")
                nc.vector.memset(n2r[:, :, :], 1.0)
                nc.sync.dma_start(out=n2r[1:2, :, :], in_=ag_out[:, 256, :])
                e2 = sing.tile([2, CL], f32, tag="e2")
                nc.vector.memset(e2[:, :], 1.0)
                nc.sync.dma_start(out=e2[0:1, :], in_=ag_in[256:257, :])

                # ---- gram matmul producing n2[c] + n2[c'] - 2*gram ----
                CA = NCORES * CL
                ps_g = psum.tile([CL, CA], f32, tag="ps_g")
                nc.tensor.matmul(
                    ps_g[:, :], lhsT=lhsT[:, 0, :],
                    rhs=rhs[:, 0, :, :].rearrange("p r c -> p (r c)"),
                    start=True, stop=False,
                )
                nc.tensor.matmul(
                    ps_g[:, :], lhsT=lhsT[:, 1, :],
                    rhs=rhs[:, 1, :, :].rearrange("p r c -> p (r c)"),
                    start=False, stop=False,
                )
                nc.tensor.matmul(
                    ps_g[:, :], lhsT=e2[:, :],
                    rhs=n2r.rearrange("two r c -> two (r c)"),
                    start=False, stop=True,
                )

                dist = sing.tile([CL, CA], f32, tag="dist")
                nc.vector.tensor_scalar(
                    out=dist, in0=ps_g[:, :], scalar1=eps12, scalar2=None,
                    op0=mybir.AluOpType.max,
                )
                nc.scalar.activation(
                    out=dist, in_=dist, func=mybir.ActivationFunctionType.Sqrt
                )
                nc.vector.tensor_reduce(
                    out=inter_rows, in_=dist,
                    axis=mybir.AxisListType.X, op=mybir.AluOpType.add,
                )

            # ---- output ----
            nc.vector.tensor_copy(out=ov[:, 0:1], in_=intra_t)
            nc.vector.tensor_copy(out=ov[:, 1:2], in_=inter_rows)
            nc.sync.dma_start(out=out_vals[:, :], in_=ov)

    nc.finalize()
    return nc


def _build_inmaps(src_x, tgt_x, src_y, tgt_y, queue, qs, assign, lmax, offs, tot):
    mmdt = _mm_dtype_np()
    # pre-tiled x: [128, 32, 256], shared across cores
    xs_t = np.ascontiguousarray(
        src_x.reshape(N // 128, 128, F).transpose(1, 0, 2).astype(mmdt)
    )
    xt_t = np.ascontiguousarray(
        tgt_x.reshape(N // 128, 128, F).transpose(1, 0, 2).astype(mmdt)
    )
    in_maps = []
    for i in range(NCORES):
        cols = assign[:, i]
        qpack = np.zeros((F, tot), dtype=np.float32)
        ysh = np.zeros((N, CL), dtype=np.float32)
        tsh = np.zeros((N, CL), dtype=np.float32)
        numv = np.zeros((CL, 1), dtype=np.float32)
        ivb = np.zeros((CL, 1), dtype=np.float32)
        for j, c in enumerate(cols):
            if c < 0:
                ivb[j, 0] = NEG_BIG
                continue
            L = int(qs[c])
            qpack[:, offs[j] : offs[j] + L] = queue[c, :, 0:L]
            ysh[:, j] = src_y[:, c]
            tsh[:, j] = tgt_y[:, c]
            numv[j, 0] = float(L)
        ys_t = np.ascontiguousarray(
            ysh.reshape(N // 128, 128, CL).transpose(1, 0, 2).astype(mmdt)
        )
        yt_t = np.ascontiguousarray(
            tsh.reshape(N // 128, 128, CL).transpose(1, 0, 2).astype(mmdt)
        )
        in_maps.append(
            {
                "qpack": qpack,
                "ysrc": ys_t,
                "ytgt": yt_t,
                "xsrc": xs_t,
                "xtgt": xt_t,
                "numcol": numv,
                "invbias": ivb,
            }
        )
    return in_maps


def kernel(src_x, tgt_x, src_y, tgt_y, queue, queue_size):
    global LAST_EXEC_NS
    _install_ntff_shim()
    from concourse.bass_utils import run_bass_kernel_spmd

    src_x = np.asarray(src_x, dtype=np.float32)
    tgt_x = np.asarray(tgt_x, dtype=np.float32)
    src_y = np.asarray(src_y, dtype=np.float32)
    tgt_y = np.asarray(tgt_y, dtype=np.float32)
    queue = np.asarray(queue, dtype=np.float32)
    qs = np.asarray(queue_size).astype(np.int64)

    assign, lmax, offs, tot, chunks, zero_start = _plan(qs)
    nc = _build_program(lmax, offs, tot, chunks, zero_start)
    in_maps = _build_inmaps(
        src_x, tgt_x, src_y, tgt_y, queue, qs, assign, lmax, offs, tot
    )

    trace = os.environ.get("CDD_KERNEL_TRACE", "0") == "1"
    kwargs = {}
    if trace:
        kwargs["trace"] = True
        tdir = os.environ.get("CDD_KERNEL_TRACE_DIR")
        if tdir:
            os.makedirs(tdir, exist_ok=True)
            kwargs["tmpdir"] = tdir
    res = run_bass_kernel_spmd(
        nc, in_maps, core_ids=list(range(NCORES)), **kwargs
    )
    LAST_EXEC_NS = res.exec_time_ns

    intra_sum = 0.0
    inter_sum = 0.0
    for i in range(NCORES):
        ov = res.results[i]["out_vals"]  # [CL, 2]
        for j, c in enumerate(assign[:, i]):
            if c >= 0:
                intra_sum += float(ov[j, 0])
                inter_sum += float(ov[j, 1])
    intra = np.float32(intra_sum / C)
    inter = np.float32(inter_sum / (C * C))
    return intra, inter
